# revision 18
# baseline (speedup 1.0000x reference)
"""Disentangled multi-head attention (DeBERTa-style) on 8 Trainium2 NeuronCores.

Sharding: core c -> batch b = c // 4, head group g = c % 4 (4 of 16 heads).
Each core computes its 4 heads end-to-end (column-parallel QKV projections,
attention, row-parallel slice of the output projection); the host sums the
4 partial outputs per batch in fp32 and adds the bias terms.

Math folds (exact up to bf16 rounding):
  - scores = (q_c.(k_c+k_p) + q_p.k_c) * s as ONE K=128 matmul per tile with
    per-head channel layout [qc*s; qp*s] vs [kc+kp; kc] (scale folded into
    weights).  ODD heads use the flipped layout [qp; qc] vs [kc; kcp] so that
    head-PAIR-packed projection matmuls produce partition-aligned PSUM->SBUF
    copies.
  - q/k/v projections run COMPENSATED fp8 DoubleRow: x = x8 + dx8 and
    W = w8 + dw8 (residuals quantized to fp8 again); the three products
    x8.w8 + x8.dw8 + dx8.w8 accumulate in PSUM (error ~dx.dw ~ 0.2%, below
    bf16 noise) at 0.75x the bf16 PE cost (12 DoubleRow kb-pair matmuls at
    0.5 cyc/col vs 8 bf16 matmuls at 1.0).
  - gate: computed as per-q COLUMNS (16 N=1 matmuls into a PSUM corner)
    instead of partition-broadcast rows; tanh REPLACES sigmoid (tanh shares
    the ACT table with exp -> no 1283ns table reloads):
    sigmoid(z) = 0.5*(1+tanh(z/2)); the 0.5 is folded into spatial_bias on
    the host.  dg diag blocks = ident * (tanh+1) via per-partition
    tensor_scalar (DVE 4x mode).
  - gate*spatial_bias accumulated into the score PSUM via 4 sub-matmuls
    lhsT=sb[qblock, kblock] and rhs=diag(gate[qblock]).
  - softmax without max-subtraction (scores bounded ~+-8, fp32-exp safe).
  - row-sums from a ones-column matmul sharing the et stationary.
  - transposes run bf16 (1 cyc/row vs 2 for fp32); bv and bo folded on host
    (softmax rows sum to 1 when mask is all-True).

Engine placement: ACT does exp (and the tiny [128,8] tanh) ONLY during the
attention phase; copies/normalization ride DVE.  Scheduling: DMA issue
order = consumption order (HWDGE is in-order); x/sb streams prefetched one
phase ahead; PE pre-warmed to ramp the p-state; transposes and the
out-projection ride as fillers inside the attention j-loop.
"""

import sys

sys.path.insert(0, "/opt/trn_rl_repo")

from contextlib import ExitStack

import numpy as np
import ml_dtypes

import concourse.bass as bass
from concourse import mybir, masks
from concourse.tile import TileContext
from concourse.bass_utils import run_bass_kernel_spmd

BF16 = ml_dtypes.bfloat16
FP8 = ml_dtypes.float8_e4m3

B, L, D = 2, 2048, 1024
H = 16
HK = 64          # head dim
NCORES = 8
HPC = 4          # heads per core
CS = HPC * HK    # channels per core = 256
NJ = L // 128    # 16 key/token blocks
NCH = L // 512   # 4 query chunks
KB_D = D // 128  # 8 contraction blocks for a 1024-deep dim
KBP = KB_D // 2  # 4 kb-pairs (DoubleRow)
SCALE = float(1.0 / np.sqrt(HK))
NWARM = 34       # PE p-state warmup matmuls

_IDENT = mybir.ActivationFunctionType.Identity

_FP32 = mybir.dt.float32
_BF16 = mybir.dt.bfloat16
_FP8 = mybir.dt.float8e4
_EXP = mybir.ActivationFunctionType.Exp
_TANH = mybir.ActivationFunctionType.Tanh
_DROW = mybir.MatmulPerfMode.DoubleRow


def _split_multiwaits(nc, skip_opcodes=()):
    """This walrus build encodes at most one sync-wait per TPB instruction.
    Tile attaches several; hoist the extras onto same-engine NoOps placed
    immediately before the instruction (engines are in-order, so semantics
    are preserved)."""
    nsplit = 0
    for fn in nc.m.functions:
        for blk in fn.blocks:
            insts = blk.instructions
            out = []
            for inst in insts:
                si = inst.sync_info
                waits = list(si.on_wait) if si is not None and si.on_wait else []
                if len(waits) > 1 and inst.opcode not in skip_opcodes:
                    si.on_wait = waits[-1:]
                    for i, w in enumerate(waits[:-1]):
                        nop = mybir.InstNoOp(name=f"{inst.name}-w{i}",
                                             ins=[], outs=[])
                        nop.engine = inst.engine
                        nop.sync_info = type(si)(on_wait=[w], on_update=[])
                        out.append(nop)
                    nsplit += 1
                out.append(inst)
            if len(out) != len(insts):
                blk.instructions = out
    return nsplit


def build_nc():
    """Emit the per-core BIR (identical on all 8 cores; data differs)."""
    nc = bass.Bass()

    # host-repacked wide layouts (see kernel() for the packing).
    # x streams carry [primary fp8 | residual fp8] halves for compensation.
    xqr = nc.dram_tensor("xqr", [128, NCH * 16384], _FP8, kind="ExternalInput")
    xkk = nc.dram_tensor("xkk", [128, 65536], _FP8, kind="ExternalInput")
    xvr = nc.dram_tensor("xvr", [128, 32768], _FP8, kind="ExternalInput")
    sbq = nc.dram_tensor("sbq", [L, L], _BF16, kind="ExternalInput")
    wqp = nc.dram_tensor("wqp", [128, 8192], _FP8, kind="ExternalInput")
    wkc2 = nc.dram_tensor("wkc2", [128, 8192], _FP8, kind="ExternalInput")
    wkk2 = nc.dram_tensor("wkk2", [128, 4096], _FP8, kind="ExternalInput")
    wvr = nc.dram_tensor("wvr", [128, 4096], _FP8, kind="ExternalInput")
    wgc = nc.dram_tensor("wgc", [128, 1], _BF16, kind="ExternalInput")
    wo = nc.dram_tensor("wo", [CS, D], _BF16, kind="ExternalInput")
    pb = nc.dram_tensor("pb", [128, 8], _FP32, kind="ExternalInput")
    g0 = nc.dram_tensor("g0", [128, 1], _FP32, kind="ExternalInput")
    outT = nc.dram_tensor("outT", [D, L], _BF16, kind="ExternalOutput")

    with TileContext(nc) as tc, ExitStack() as top:
        pool = lambda **kw: top.enter_context(tc.tile_pool(**kw))

        const_pool = pool(name="const", bufs=1)
        w_pool = pool(name="w", bufs=1)
        bias_pool = pool(name="bias", bufs=1)
        qk_pool = pool(name="qkres", bufs=1)
        v_pool = pool(name="vres", bufs=1)
        sb_pool = pool(name="sb", bufs=8)      # streamed ring
        x_pool = pool(name="xin", bufs=1)      # per-tag rings set on tile()
        gb_pool = pool(name="gb", bufs=1)
        dg_pool = pool(name="dg", bufs=1)
        e_pool = pool(name="et", bufs=3)
        csb_pool = pool(name="csb", bufs=1)
        cta_pool = pool(name="cta", bufs=1)
        oute_pool = pool(name="oute", bufs=3)

        scr = const_pool.tile([128, 512], _BF16, tag="scr", name="scr")
        nc.gpsimd.memset(scr[:], 0.0)
        ident = const_pool.tile([128, 128], _BF16, tag="ident", name="ident")
        masks.make_identity(nc, ident[:])
        ones_t = const_pool.tile([128, 1], _BF16, tag="ones", name="ones")
        nc.gpsimd.memset(ones_t[:], 1.0)

        # ---- DMA issue order = consumption order (HWDGE is in-order) -----
        wv_t = w_pool.tile([128, 2, 4, 2, 256], _FP8, tag="wv", name="wvt")
        nc.sync.dma_start(wv_t[:], wvr[:, :])
        xv_t = []
        for half in range(2):
            t = x_pool.tile([128, 2, 4, 2, 1024], _FP8, tag="xv",
                            name=f"xv{half}", bufs=2)
            nc.sync.dma_start(t[:], xvr[:, half * 16384:(half + 1) * 16384])
            xv_t.append(t)
        wkc_t = w_pool.tile([128, 2, 8, 2, 256], _FP8, tag="wkc", name="wkct")
        nc.sync.dma_start(wkc_t[:], wkc2[:, :])
        wkk_t = w_pool.tile([128, 2, 4, 2, 256], _FP8, tag="wkk", name="wkkt")
        nc.sync.dma_start(wkk_t[:], wkk2[:, :])
        wgc_t = const_pool.tile([128, 1], _BF16, tag="wgc", name="wgct")
        nc.sync.dma_start(wgc_t[:], wgc[:, :])
        pb_t = bias_pool.tile([128, 8], _FP32, tag="pb", name="pbt")
        nc.sync.dma_start(pb_t[:], pb[:, :])
        g0_t = bias_pool.tile([128, 1], _FP32, tag="g0", name="g0t")
        nc.sync.dma_start(g0_t[:], g0[:, :])

        SINV = float(1.0 / 256.0)  # undo the host fp8 pre-scales SX*SW

        def copy_bias(dst, src, bias_ap, on_act):
            """PSUM->SBUF copy: (src/256) + bias, on DVE or ACT."""
            if on_act:
                nc.scalar.activation(dst, src, _IDENT, bias=bias_ap,
                                     scale=SINV)
            else:
                nc.vector.tensor_scalar(dst, src, SINV, bias_ap,
                                        op0=mybir.AluOpType.mult,
                                        op1=mybir.AluOpType.add)

        qcat = [qk_pool.tile([128, L], _BF16, tag=f"qcat{h}", name=f"qcat{h}")
                for h in range(HPC)]
        kcat = [qk_pool.tile([128, L], _BF16, tag=f"kcat{h}", name=f"kcat{h}")
                for h in range(HPC)]
        vb4 = [None] * NJ
        sbq_t = [None] * NJ
        xq_t = [None] * NCH

        def load_sbq(rows):
            for r in rows:
                t = sb_pool.tile([128, L], _BF16, tag="sbq", name=f"sbq{r}")
                nc.sync.dma_start(t[:], sbq[r * 128:(r + 1) * 128, :])
                sbq_t[r] = t

        def load_xq(ch):
            """Two wide DMAs for p2q(ch): [q|pos_q] primary + residual."""
            tq = x_pool.tile([128, 2, 4, 2, 512], _FP8, tag="xq",
                             name=f"xq{ch}", bufs=4)
            nc.sync.dma_start(tq[:], xqr[:, ch * 16384:ch * 16384 + 8192])
            tp = x_pool.tile([128, 2, 4, 2, 512], _FP8, tag="xq",
                             name=f"xp{ch}", bufs=4)
            nc.sync.dma_start(tp[:],
                              xqr[:, ch * 16384 + 8192:(ch + 1) * 16384])
            xq_t[ch] = (tq, tp)

        # ---- PE warmup on scratch: ramp p-state while DMAs land ---------
        with tc.tile_pool(name="ps_warm", bufs=1, space="PSUM") as warm_pool:
            wmt = warm_pool.tile([128, 128], _FP32, tag="warm", name="warm")
            for i in range(NWARM):
                nc.tensor.matmul(wmt[:], scr[:, 0:128], scr[:, 0:128],
                                 start=True, stop=True)

        # compensated 3-term schedule: (x_half, w_half) in product order
        TERMS = ((0, 0), (0, 1), (1, 0))

        # ---- P1: v projection, compensated fp8 DoubleRow ----------------
        with tc.tile_pool(name="ps_v", bufs=8, space="PSUM") as psv_pool:
            for half in range(2):
                psv = [psv_pool.tile([128, CS], _FP32, tag="ps_v",
                                     name=f"psv{half}_{i}") for i in range(8)]
                xt = xv_t[half]
                for ti, (xh, wh) in enumerate(TERMS):
                    for kbp in range(KBP):
                        first = (ti == 0 and kbp == 0)
                        last = (ti == len(TERMS) - 1 and kbp == KBP - 1)
                        for i in range(8):
                            nc.tensor.matmul(
                                psv[i][:],
                                xt[:, xh, kbp, 0:2, i * 128:(i + 1) * 128],
                                wv_t[:, wh, kbp, 0:2, :],
                                start=first, stop=last, perf_mode=_DROW)
                for i in range(8):
                    tb = half * 8 + i
                    vb = v_pool.tile([128, CS], _BF16, tag=f"vb{tb}",
                                     name=f"vb{tb}")
                    nc.vector.tensor_scalar_mul(vb[:], psv[i][:], SINV)
                    vb4[tb] = vb

        # ---- P2: k-side projections, compensated DR, head-pair packed ---
        # kcat[even] = [kc+kp ; kc], kcat[odd] = [kc ; kc+kp]
        with tc.tile_pool(name="ps_k", bufs=8, space="PSUM") as psk_pool:
            for pc in range(2):  # key half (1024 keys each)
                ps1 = [[psk_pool.tile([128, 512], _FP32, tag="psk",
                                      name=f"k1_{pc}_{pr}_{i}")
                        for i in range(2)] for pr in range(2)]
                ps2 = [[psk_pool.tile([128, 512], _FP32, tag="psk",
                                      name=f"k2_{pc}_{pr}_{i}")
                        for i in range(2)] for pr in range(2)]
                for p in range(8):  # kb-pairs over the 2048-deep k||pos_k
                    t = x_pool.tile([128, 2, 2, 1024], _FP8, tag="xkk",
                                    name=f"xkk{pc}_{p}", bufs=5)
                    nc.sync.dma_start(
                        t[:], xkk[:, (pc * 8 + p) * 4096:
                                  (pc * 8 + p + 1) * 4096])
                    for pr in range(2):
                        psl = slice(pr * 128, (pr + 1) * 128)
                        for i in range(2):
                            isl = slice(i * 512, (i + 1) * 512)
                            for ti, (xh, wh) in enumerate(TERMS):
                                nc.tensor.matmul(
                                    ps1[pr][i][:],
                                    wkc_t[:, wh, p, 0:2, psl],
                                    t[:, xh, 0:2, isl],
                                    start=(p == 0 and ti == 0),
                                    stop=(p == 7 and ti == len(TERMS) - 1),
                                    perf_mode=_DROW)
                                if p < 4:
                                    nc.tensor.matmul(
                                        ps2[pr][i][:],
                                        wkk_t[:, wh, p, 0:2, psl],
                                        t[:, xh, 0:2, isl],
                                        start=(p == 0 and ti == 0),
                                        stop=(p == 3 and
                                              ti == len(TERMS) - 1),
                                        perf_mode=_DROW)
                if pc == 0:
                    wqp_t = w_pool.tile([128, 2, 4, 2, 2, 256], _FP8,
                                        tag="wqp", name="wqpt")
                    nc.sync.dma_start(wqp_t[:], wqp[:, :])
                for pr in range(2):
                    he, ho = 2 * pr, 2 * pr + 1
                    c = 4 + 2 * pr
                    for i in range(2):
                        csl = slice(pc * 1024 + i * 512,
                                    pc * 1024 + (i + 1) * 512)
                        on_act = i == 0
                        copy_bias(kcat[he][0:64, csl], ps1[pr][i][0:64, :],
                                  pb_t[0:64, c:c + 1], on_act)
                        copy_bias(kcat[ho][64:128, csl], ps1[pr][i][64:128, :],
                                  pb_t[64:128, c:c + 1], on_act)
                        copy_bias(kcat[ho][0:64, csl], ps2[pr][i][0:64, :],
                                  pb_t[0:64, c + 1:c + 2], not on_act)
                        copy_bias(kcat[he][64:128, csl], ps2[pr][i][64:128, :],
                                  pb_t[64:128, c + 1:c + 2], not on_act)

        load_xq(0)
        load_sbq(range(0, 4))
        wo_t = w_pool.tile([128, D], _BF16, tag="wo", name="wot")
        nc.sync.dma_start(wo_t[:], wo[0:128, :])
        wo_t2 = w_pool.tile([128, D], _BF16, tag="wo2", name="wot2")
        nc.sync.dma_start(wo_t2[:], wo[128:256, :])
        wo_ts = [wo_t, wo_t2]

        # ---- shared PSUM pools (8 banks exactly) -------------------------
        ps2_pool = pool(name="ps2", bufs=3, space="PSUM")
        pcv_pool = pool(name="ps_cv", bufs=1, space="PSUM")   # ctx accum
        pcs_pool = pool(name="ps_cs", bufs=1, space="PSUM")   # rowsums

        # ---- per-chunk pipeline ------------------------------------------
        def p2q_pair(ch, pr):
            """q-side projections for chunk ch, head pair pr, followed by the
            pair's column-gate/diag so A can start as soon as pair 0 lands."""
            csl = slice(ch * 512, (ch + 1) * 512)
            prsl = slice(pr * 128, (pr + 1) * 128)
            tq, tp = xq_t[ch]
            d = ps2_pool.tile([128, 1024], _FP32, tag="ps2", name=f"q{ch}{pr}")
            mq, mp = d[:, 0:512], d[:, 512:1024]
            for ti, (xh, wh) in enumerate(TERMS):
                for kbp in range(KBP):
                    st = (ti == 0 and kbp == 0)
                    sp = (ti == len(TERMS) - 1 and kbp == KBP - 1)
                    nc.tensor.matmul(mq, wqp_t[:, wh, kbp, 0, 0:2, prsl],
                                     tq[:, xh, kbp, 0:2, :],
                                     start=st, stop=sp, perf_mode=_DROW)
                    nc.tensor.matmul(mp, wqp_t[:, wh, kbp, 1, 0:2, prsl],
                                     tp[:, xh, kbp, 0:2, :],
                                     start=st, stop=sp, perf_mode=_DROW)
            he, ho = 2 * pr, 2 * pr + 1
            c = 2 * pr
            copy_bias(qcat[he][0:64, csl], d[0:64, 0:512],
                      pb_t[0:64, c:c + 1], False)
            copy_bias(qcat[ho][64:128, csl], d[64:128, 0:512],
                      pb_t[64:128, c:c + 1], False)
            copy_bias(qcat[ho][0:64, csl], d[0:64, 512:1024],
                      pb_t[0:64, c + 1:c + 2], False)
            copy_bias(qcat[he][64:128, csl], d[64:128, 512:1024],
                      pb_t[64:128, c + 1:c + 2], False)
            # column gate: 8 N=1 matmuls into a PSUM corner (free on PE),
            # one tiny tanh, diag blocks via per-partition tensor_scalar.
            for hh, h in enumerate((he, ho)):
                rows = slice(0, 64) if h % 2 == 0 else slice(64, 128)
                for qb in range(4):
                    nc.tensor.matmul(
                        d[:, hh * 4 + qb:hh * 4 + qb + 1],
                        qcat[h][rows, ch * 512 + qb * 128:
                                ch * 512 + (qb + 1) * 128],
                        wgc_t[rows, 0:1], start=True, stop=True)
            g = gb_pool.tile([128, 8], _BF16, tag=f"gb{pr}",
                             name=f"gb{pr}_{ch}")
            # gate' = 1 + tanh(z/2 + bg/2) = 2*sigmoid(z+bg); 0.5 folded
            # into spatial_bias host-side.
            nc.scalar.activation(g[:], d[:, 0:8], _TANH, bias=g0_t[:, 0:1],
                                 scale=0.5)
            gp1 = gb_pool.tile([128, 8], _FP32, tag=f"gp{pr}",
                               name=f"gp{pr}_{ch}")
            nc.vector.tensor_scalar_add(gp1[:], g[:], 1.0)
            dg = []
            for hh, h in enumerate((he, ho)):
                ds = []
                for sb in range(4):
                    d2 = dg_pool.tile([128, 128], _BF16, tag=f"dg{h}_{sb}",
                                      name=f"dg{h}_{sb}_{ch}")
                    nc.vector.tensor_scalar_mul(
                        d2[:], ident[:], gp1[:, hh * 4 + sb:hh * 4 + sb + 1])
                    ds.append(d2)
                dg.append(ds)
            return dg

        def attn_pass(ch, hp, dg, pcs, fillers=()):
            """Score + gated spatial bias + exp + ctx/rowsum accumulation
            for ONE head pair; `fillers` are small PE work units emitted one
            per j iteration to absorb the ACT exp-lag bubbles."""
            if hp == 0 and ch + 1 < NCH:
                load_xq(ch + 1)
                load_sbq(range(4 * (ch + 1), 4 * (ch + 2)))
            dsl = slice(ch * 512, (ch + 1) * 512)
            pcv = pcv_pool.tile([128, 512], _FP32, tag="ps_cv",
                                name=f"pcv{hp}_{ch}")
            fillers = list(fillers)
            for j in range(NJ):
                if j > 0 and fillers:
                    fillers.pop(0)()
                jsl = slice(j * 128, (j + 1) * 128)
                d = ps2_pool.tile([128, 1024], _FP32, tag="ps2", name="pss")
                for hh in range(2):
                    h = 2 * hp + hh
                    hof = hh * 512
                    nc.tensor.matmul(d[:, hof:hof + 512],
                                     kcat[h][:, jsl], qcat[h][:, dsl],
                                     start=True, stop=False)
                    for sb in range(4):
                        nc.tensor.matmul(
                            d[:, hof + sb * 128:hof + (sb + 1) * 128],
                            sbq_t[ch * 4 + sb][:, jsl],
                            dg[h - 2 * hp][sb][:],
                            start=False, stop=True)
                if j == 0:
                    # zero the accumulators AFTER j0's scores so the pass is
                    # not head-blocked on the previous pair's norm_div
                    nc.tensor.matmul(pcv[:], scr[:, 0:128], scr[:, 0:512],
                                     start=True, stop=False)
                    if hp == 0:
                        nc.tensor.matmul(pcs[:], scr[:, 0:128],
                                         scr[:, 0:16], start=True,
                                         stop=False)
                et = e_pool.tile([128, 1024], _BF16, tag="et", name="ett")
                nc.scalar.activation(et[:], d[:], _EXP)
                for hh in range(2):
                    h = 2 * hp + hh
                    for s in range(4):
                        esl = et[:, hh * 512 + s * 128:
                                 hh * 512 + (s + 1) * 128]
                        nc.tensor.matmul(
                            pcv[:, hh * 256 + s * 64:hh * 256 + (s + 1) * 64],
                            esl, vb4[j][:, h * 64:(h + 1) * 64],
                            start=False, stop=(j == NJ - 1))
                        nc.tensor.matmul(
                            pcs[:, h * 4 + s:h * 4 + s + 1],
                            esl, ones_t[:],
                            start=False, stop=(j == NJ - 1))
            for u in fillers:  # flush any unconsumed work units
                u()
            return pcv

        def norm_div(ch, pr, pcv, pcs):
            """Normalize one pair's ctx by its rowsums; frees pcv's bank."""
            inv = csb_pool.tile([128, 8], _FP32, tag=f"inv{pr}",
                                name=f"inv{pr}_{ch}")
            nc.vector.reciprocal(inv[:], pcs[:, pr * 8:(pr + 1) * 8])
            csb2 = {}
            for s in range(4):
                t = csb_pool.tile([128, 128], _BF16, tag=f"cs{pr}_{s}",
                                  name=f"cs{pr}_{s}_{ch}")
                for hh in range(2):
                    nc.vector.tensor_scalar_mul(
                        t[:, hh * 64:(hh + 1) * 64],
                        pcv[:, hh * 256 + s * 64:hh * 256 + (s + 1) * 64],
                        inv[:, hh * 4 + s:hh * 4 + s + 1])
                csb2[s] = t
            return csb2

        def cta_tile(ch, pr):
            return cta_pool.tile([128, 512], _BF16, tag=f"cta{pr}",
                                 name=f"cta{pr}_{ch}")

        def t_units(csb2, cta):
            """Transposes for one pair, 2 per ring double (one per bank)."""
            units = []

            def t_unit(s2):
                def go():
                    d16 = ps2_pool.tile([128, 2048], _BF16, tag="ps2",
                                        name="ptt")
                    for hh in range(2):
                        s = 2 * s2 + hh
                        nc.tensor.matmul(d16[:, hh * 1024:hh * 1024 + 128],
                                         csb2[s][:], ident[:],
                                         is_transpose=True)
                    for hh in range(2):
                        s = 2 * s2 + hh
                        nc.vector.tensor_copy(
                            cta[:, s * 128:(s + 1) * 128],
                            d16[:, hh * 1024:hh * 1024 + 128])
                return go

            for s2 in range(2):
                units.append(t_unit(s2))
            return units

        def o_units(ch, cta01):
            units = []

            def o_unit(op):
                def go():
                    d = ps2_pool.tile([128, 1024], _FP32, tag="ps2",
                                      name=f"o{ch}{op}")
                    for hh in range(2):
                        ob = 2 * op + hh
                        for kb in range(2):
                            nc.tensor.matmul(
                                d[:, hh * 512:(hh + 1) * 512],
                                wo_ts[kb][:, ob * 128:(ob + 1) * 128],
                                cta01[kb][:], start=(kb == 0), stop=(kb == 1))
                    ot = oute_pool.tile([128, 1024], _BF16, tag="ot",
                                        name="ott", bufs=2)
                    nc.vector.tensor_copy(ot[:], d[:])
                    for hh in range(2):
                        ob = 2 * op + hh
                        nc.sync.dma_start(
                            outT[ob * 128:(ob + 1) * 128,
                                 ch * 512:(ch + 1) * 512],
                            ot[:, hh * 512:(hh + 1) * 512])
                return go

            for op in range(KB_D // 2):
                units.append(o_unit(op))
            return units

        # pipeline: passA(ch) absorbs prev chunk's pair-1 transposes + O;
        # passB(ch) absorbs this chunk's pair-0 transposes.
        prev = None          # (ch, pcv_pair1, pcs, cta_pair0)
        for ch in range(NCH):
            dg0 = p2q_pair(ch, 0)
            fillA = []
            if prev is not None:
                pch, ppcv1, ppcs, pcta0 = prev
                csb2p1 = norm_div(pch, 1, ppcv1, ppcs)
            dg1 = p2q_pair(ch, 1)
            if prev is not None:
                pcta1 = cta_tile(pch, 1)
                fillA = t_units(csb2p1, pcta1) + o_units(pch, [pcta0, pcta1])
            pcs = pcs_pool.tile([128, 16], _FP32, tag="pcs", name=f"pcs{ch}")
            pcv0 = attn_pass(ch, 0, dg0, pcs, fillA)
            csb2a = norm_div(ch, 0, pcv0, pcs)
            cta0 = cta_tile(ch, 0)
            pcv1 = attn_pass(ch, 1, dg1, pcs, t_units(csb2a, cta0))
            prev = (ch, pcv1, pcs, cta0)
        pch, ppcv1, ppcs, pcta0 = prev
        csb2p1 = norm_div(pch, 1, ppcv1, ppcs)
        pcta1 = cta_tile(pch, 1)
        for u in t_units(csb2p1, pcta1) + o_units(pch, [pcta0, pcta1]):
            u()

    _split_multiwaits(nc)
    return nc


_NC_CACHE = {}


def _get_nc():
    if "nc" not in _NC_CACHE:
        _NC_CACHE["nc"] = build_nc()
    return _NC_CACHE["nc"]


def _np_reference(k, v, q, mask, spatial_bias, pos_k, pos_q,
                  Wk, bk, Wv, bv, Wq, bq, Wpk, bpk, Wpq, bpq, Wo, bo, Wg, bg):
    """Slow numpy fallback (only if mask is not all-True)."""
    def lin(x, W, b):
        return x @ W.T + b

    def split(x):
        return x.reshape(B, L, H, -1).transpose(0, 2, 1, 3)

    k_c, v_c, q_c = split(lin(k, Wk, bk)), split(lin(v, Wv, bv)), split(lin(q, Wq, bq))
    k_p, q_p = split(lin(pos_k, Wpk, bpk)), split(lin(pos_q, Wpq, bpq))
    scores = (np.einsum("bhqd,bhkd->bhqk", q_c, k_c)
              + np.einsum("bhqd,bhkd->bhqk", q_c, k_p)
              + np.einsum("bhqd,bhkd->bhqk", q_p, k_c)) * SCALE
    gate = 1.0 / (1.0 + np.exp(-(q_c @ Wg.T + bg)))
    scores = scores + gate * spatial_bias
    scores = np.where(mask[:, None, :, :], scores, -np.inf)
    scores = scores - scores.max(-1, keepdims=True)
    e = np.exp(scores)
    attn = e / e.sum(-1, keepdims=True)
    ctx = np.einsum("bhqk,bhkd->bhqd", attn, v_c)
    ctx = ctx.transpose(0, 2, 1, 3).reshape(B, L, D)
    return lin(ctx, Wo, bo).astype(np.float32)


def _pairstack8(w, scale):
    """[NP*256, M] fp32 -> ([128, NP*2*M] fp8 primary, same-shape residual):
    sub-major stacking of 128-row block PAIRS (DoubleRow K-tile layout).
    `scale` lifts the values out of fp8's subnormal floor so the residual
    (re-quantized to fp8) stays ~4% of a 4% correction; the joint x*w scale
    (SX*SW=256) is divided out in the PSUM->SBUF copies."""
    w = w * scale
    w8 = w.astype(FP8)
    dw8 = (w - w8.astype(np.float32)).astype(FP8)
    npair = w.shape[0] // 256
    m = w.shape[1]

    def stk(a):
        blocks = []
        for p in range(npair):
            blocks.append(np.stack(
                [a[(2 * p) * 128:(2 * p + 1) * 128],
                 a[(2 * p + 1) * 128:(2 * p + 2) * 128]],
                axis=1).reshape(128, 2 * m))
        return np.ascontiguousarray(np.concatenate(blocks, axis=1))

    return stk(w8), stk(dw8)


SX = 4.0    # x-stream fp8 pre-scale
SW = 64.0   # weight fp8 pre-scale
SINV = float(1.0 / (SX * SW))


def _comp(w, scale):
    """[NP*256, M] -> [128, 2*NP*2*M] fp8: primary block then residual."""
    a, b = _pairstack8(w, scale)
    return np.ascontiguousarray(np.concatenate([a, b], axis=1))


def kernel(k, v, q, mask, spatial_bias, pos_k, pos_q,
           Wk, bk, Wv, bv, Wq, bq, Wpk, bpk, Wpq, bpq, Wo, bo, Wg, bg,
           **_unused):
    f32 = lambda x: np.asarray(x, np.float32)
    k, v, q, pos_k, pos_q = f32(k), f32(v), f32(q), f32(pos_k), f32(pos_q)
    spatial_bias = f32(spatial_bias)
    mask = np.asarray(mask)
    Wk, Wv, Wq, Wpk, Wpq, Wo, Wg = map(f32, (Wk, Wv, Wq, Wpk, Wpq, Wo, Wg))
    bk, bv, bq, bpk, bpq, bo, bg = map(f32, (bk, bv, bq, bpk, bpq, bo, bg))

    if not mask.all():
        return _np_reference(k, v, q, mask, spatial_bias, pos_k, pos_q,
                             Wk, bk, Wv, bv, Wq, bq, Wpk, bpk, Wpq, bpq,
                             Wo, bo, Wg, bg)

    nc = _get_nc()

    xqr_b, xkk_b, xvr_b, sbq_b = [], [], [], []
    for b in range(B):
        qT, pT = q[b].T, pos_q[b].T                 # [D, L] fp32
        chunks = []
        for ch in range(NCH):
            csl = slice(ch * 512, (ch + 1) * 512)
            chunks.append(_comp(qT[:, csl], SX))
            chunks.append(_comp(pT[:, csl], SX))
        xqr_b.append(np.ascontiguousarray(np.concatenate(chunks, axis=1)))
        kstack = np.vstack([k[b].T, pos_k[b].T])    # [2D, L]
        xkk_b.append(np.ascontiguousarray(np.concatenate(
            [_xkk_interleave(kstack[:, pc * 1024:(pc + 1) * 1024])
             for pc in range(2)], axis=1)))
        vT = v[b].T
        xvr_b.append(np.ascontiguousarray(np.concatenate(
            [_comp(vT[:, half * 1024:(half + 1) * 1024], SX)
             for half in range(2)], axis=1)))
        # 0.5 folded from the tanh-gate: gate = 0.5*(1+tanh((z+bg)/2))
        sbq_b.append(np.ascontiguousarray(
            spatial_bias[b, 0] * 0.5).astype(BF16))

    WqT, WpqT = Wq.T * SCALE, Wpq.T * SCALE
    WkT, WpkT, WvT, WoT = Wk.T, Wpk.T, Wv.T, Wo.T
    in_maps = []
    for c in range(NCORES):
        b, g = c // 4, c % 4
        cs = slice(g * CS, (g + 1) * CS)

        def chs(hl):  # global channel slice for local head hl
            h = 4 * g + hl
            return slice(h * HK, (h + 1) * HK)

        wq2_a = np.empty((D, 256), np.float32)
        wp2_a = np.empty((D, 256), np.float32)
        wkc_a = np.empty((2 * D, 256), np.float32)
        wkk_a = np.empty((D, 256), np.float32)
        pb_a = np.empty((128, 8), np.float32)
        for pr in range(2):
            he, ho = 2 * pr, 2 * pr + 1
            pc = slice(pr * 128, pr * 128 + 64)
            pc2 = slice(pr * 128 + 64, pr * 128 + 128)
            wq2_a[:, pc] = WqT[:, chs(he)]
            wq2_a[:, pc2] = WqT[:, chs(ho)]
            wp2_a[:, pc] = WpqT[:, chs(ho)]      # swapped
            wp2_a[:, pc2] = WpqT[:, chs(he)]
            wkc_a[0:D, pc] = WkT[:, chs(he)]
            wkc_a[D:, pc] = WpkT[:, chs(he)]
            wkc_a[0:D, pc2] = WkT[:, chs(ho)]
            wkc_a[D:, pc2] = WpkT[:, chs(ho)]
            wkk_a[:, pc] = WkT[:, chs(ho)]       # swapped
            wkk_a[:, pc2] = WkT[:, chs(he)]
            # per-partition biases for the half-copies
            c0 = 2 * pr
            pb_a[0:64, c0] = bq[chs(he)] * SCALE
            pb_a[64:128, c0] = bq[chs(ho)] * SCALE
            pb_a[0:64, c0 + 1] = bpq[chs(ho)] * SCALE
            pb_a[64:128, c0 + 1] = bpq[chs(he)] * SCALE
            c4 = 4 + 2 * pr
            pb_a[0:64, c4] = (bk + bpk)[chs(he)]
            pb_a[64:128, c4] = (bk + bpk)[chs(ho)]
            pb_a[0:64, c4 + 1] = bk[chs(ho)]
            pb_a[64:128, c4 + 1] = bk[chs(he)]
        # wqp stripes: per (w_half, kb-pair): [qc pair block | qp pair block]
        wq8, dwq8 = _pairstack8(wq2_a, SW)    # [128, 4*512] each
        wp8, dwp8 = _pairstack8(wp2_a, SW)
        wqp_l = []
        for wh, (aq, ap_) in enumerate(((wq8, wp8), (dwq8, dwp8))):
            for p in range(KBP):
                wqp_l.append(aq[:, p * 512:(p + 1) * 512])
                wqp_l.append(ap_[:, p * 512:(p + 1) * 512])
        wqp_a = np.ascontiguousarray(np.concatenate(wqp_l, axis=1))
        in_maps.append({
            "xqr": xqr_b[b], "xkk": xkk_b[b], "xvr": xvr_b[b],
            "sbq": sbq_b[b],
            "wqp": wqp_a,
            "wkc2": _comp(wkc_a, SW),
            "wkk2": _comp(wkk_a, SW),
            "wvr": _comp(np.ascontiguousarray(WvT[:, cs]), SW),
            "wgc": np.tile((Wg[0] * (1.0 / SCALE)).astype(BF16), 2)
                     .reshape(128, 1),
            "wo": np.ascontiguousarray(WoT[cs, :]).astype(BF16),
            "pb": pb_a,
            "g0": np.full((128, 1), float(bg[0]) * 0.5, np.float32),
        })

    res = run_bass_kernel_spmd(nc, in_maps, core_ids=list(range(NCORES)))

    const_row = (bv @ WoT + bo).astype(np.float32)  # exact bv/bo fold
    out = np.empty((B, L, D), np.float32)
    for b in range(B):
        acc = res.results[b * 4]["outT"].astype(np.float32, copy=True)
        for g in range(1, 4):
            acc += res.results[b * 4 + g]["outT"]
        out[b] = acc.T + const_row
    return out


def _xkk_interleave(kst):
    """[2048, 1024] -> [128, 16384] fp8: per kb-pair p: [primary 2048 |
    residual 2048] so each stream tile holds both compensation halves."""
    a, r = _pairstack8(kst, SX)   # [128, 8*2048] each
    blocks = []
    for p in range(8):
        blocks.append(a[:, p * 2048:(p + 1) * 2048])
        blocks.append(r[:, p * 2048:(p + 1) * 2048])
    return np.ascontiguousarray(np.concatenate(blocks, axis=1))


# revision 19
# speedup vs baseline: 1.0019x; 1.0019x over previous
"""Disentangled multi-head attention (DeBERTa-style) on 8 Trainium2 NeuronCores.

Sharding: core c -> batch b = c // 4, head group g = c % 4 (4 of 16 heads).
Each core computes its 4 heads end-to-end (column-parallel QKV projections,
attention, row-parallel slice of the output projection); the host sums the
4 partial outputs per batch in fp32 and adds the bias terms.

Math folds (exact up to bf16 rounding):
  - scores = (q_c.(k_c+k_p) + q_p.k_c) * s as ONE K=128 matmul per tile with
    per-head channel layout [qc*s; qp*s] vs [kc+kp; kc] (scale folded into
    weights).  ODD heads use the flipped layout [qp; qc] vs [kc; kcp] so that
    head-PAIR-packed projection matmuls produce partition-aligned PSUM->SBUF
    copies.
  - q/k/v projections run COMPENSATED fp8 DoubleRow: x = x8 + dx8 and
    W = w8 + dw8 (residuals quantized to fp8 again); the three products
    x8.w8 + x8.dw8 + dx8.w8 accumulate in PSUM (error ~dx.dw ~ 0.2%, below
    bf16 noise) at 0.75x the bf16 PE cost (12 DoubleRow kb-pair matmuls at
    0.5 cyc/col vs 8 bf16 matmuls at 1.0).
  - gate: computed as per-q COLUMNS (16 N=1 matmuls into a PSUM corner)
    instead of partition-broadcast rows; tanh REPLACES sigmoid (tanh shares
    the ACT table with exp -> no 1283ns table reloads):
    sigmoid(z) = 0.5*(1+tanh(z/2)); the 0.5 is folded into spatial_bias on
    the host.  dg diag blocks = ident * (tanh+1) via per-partition
    tensor_scalar (DVE 4x mode).
  - gate*spatial_bias accumulated into the score PSUM via 4 sub-matmuls
    lhsT=sb[qblock, kblock] and rhs=diag(gate[qblock]).
  - softmax without max-subtraction (scores bounded ~+-8, fp32-exp safe).
  - row-sums from a ones-column matmul sharing the et stationary.
  - transposes run bf16 (1 cyc/row vs 2 for fp32); bv and bo folded on host
    (softmax rows sum to 1 when mask is all-True).

Engine placement: ACT does exp (and the tiny [128,8] tanh) ONLY during the
attention phase; copies/normalization ride DVE.  Scheduling: DMA issue
order = consumption order (HWDGE is in-order); x/sb streams prefetched one
phase ahead; PE pre-warmed to ramp the p-state; transposes and the
out-projection ride as fillers inside the attention j-loop.
"""

import sys

sys.path.insert(0, "/opt/trn_rl_repo")

from contextlib import ExitStack

import numpy as np
import ml_dtypes

import concourse.bass as bass
from concourse import mybir, masks
from concourse.tile import TileContext
from concourse.bass_utils import run_bass_kernel_spmd

BF16 = ml_dtypes.bfloat16
FP8 = ml_dtypes.float8_e4m3

B, L, D = 2, 2048, 1024
H = 16
HK = 64          # head dim
NCORES = 8
HPC = 4          # heads per core
CS = HPC * HK    # channels per core = 256
NJ = L // 128    # 16 key/token blocks
NCH = L // 512   # 4 query chunks
KB_D = D // 128  # 8 contraction blocks for a 1024-deep dim
KBP = KB_D // 2  # 4 kb-pairs (DoubleRow)
SCALE = float(1.0 / np.sqrt(HK))
NWARM = 34       # PE p-state warmup matmuls

_IDENT = mybir.ActivationFunctionType.Identity

_FP32 = mybir.dt.float32
_BF16 = mybir.dt.bfloat16
_FP8 = mybir.dt.float8e4
_EXP = mybir.ActivationFunctionType.Exp
_TANH = mybir.ActivationFunctionType.Tanh
_DROW = mybir.MatmulPerfMode.DoubleRow


def _split_multiwaits(nc, skip_opcodes=()):
    """This walrus build encodes at most one sync-wait per TPB instruction.
    Tile attaches several; hoist the extras onto same-engine NoOps placed
    immediately before the instruction (engines are in-order, so semantics
    are preserved)."""
    nsplit = 0
    for fn in nc.m.functions:
        for blk in fn.blocks:
            insts = blk.instructions
            out = []
            for inst in insts:
                si = inst.sync_info
                waits = list(si.on_wait) if si is not None and si.on_wait else []
                if len(waits) > 1 and inst.opcode not in skip_opcodes:
                    si.on_wait = waits[-1:]
                    for i, w in enumerate(waits[:-1]):
                        nop = mybir.InstNoOp(name=f"{inst.name}-w{i}",
                                             ins=[], outs=[])
                        nop.engine = inst.engine
                        nop.sync_info = type(si)(on_wait=[w], on_update=[])
                        out.append(nop)
                    nsplit += 1
                out.append(inst)
            if len(out) != len(insts):
                blk.instructions = out
    return nsplit


def build_nc():
    """Emit the per-core BIR (identical on all 8 cores; data differs)."""
    nc = bass.Bass()

    # host-repacked wide layouts (see kernel() for the packing).
    # x streams carry [primary fp8 | residual fp8] halves for compensation.
    xqr = nc.dram_tensor("xqr", [128, NCH * 16384], _FP8, kind="ExternalInput")
    xkk = nc.dram_tensor("xkk", [128, 65536], _FP8, kind="ExternalInput")
    xvr = nc.dram_tensor("xvr", [128, 32768], _FP8, kind="ExternalInput")
    sbq = nc.dram_tensor("sbq", [L, L], _BF16, kind="ExternalInput")
    wqp = nc.dram_tensor("wqp", [128, 8192], _FP8, kind="ExternalInput")
    wkc2 = nc.dram_tensor("wkc2", [128, 8192], _FP8, kind="ExternalInput")
    wkk2 = nc.dram_tensor("wkk2", [128, 4096], _FP8, kind="ExternalInput")
    wvr = nc.dram_tensor("wvr", [128, 4096], _FP8, kind="ExternalInput")
    wgc = nc.dram_tensor("wgc", [128, 1], _BF16, kind="ExternalInput")
    wo = nc.dram_tensor("wo", [CS, D], _BF16, kind="ExternalInput")
    pb = nc.dram_tensor("pb", [128, 8], _FP32, kind="ExternalInput")
    g0 = nc.dram_tensor("g0", [128, 1], _FP32, kind="ExternalInput")
    outT = nc.dram_tensor("outT", [D, L], _BF16, kind="ExternalOutput")

    with TileContext(nc) as tc, ExitStack() as top:
        pool = lambda **kw: top.enter_context(tc.tile_pool(**kw))

        const_pool = pool(name="const", bufs=1)
        w_pool = pool(name="w", bufs=1)
        bias_pool = pool(name="bias", bufs=1)
        qk_pool = pool(name="qkres", bufs=1)
        v_pool = pool(name="vres", bufs=1)
        sb_pool = pool(name="sb", bufs=8)      # streamed ring
        x_pool = pool(name="xin", bufs=1)      # per-tag rings set on tile()
        gb_pool = pool(name="gb", bufs=1)
        dg_pool = pool(name="dg", bufs=1)
        e_pool = pool(name="et", bufs=3)
        csb_pool = pool(name="csb", bufs=1)
        cta_pool = pool(name="cta", bufs=1)
        oute_pool = pool(name="oute", bufs=3)

        scr = const_pool.tile([128, 512], _BF16, tag="scr", name="scr")
        nc.gpsimd.memset(scr[:], 0.0)
        ident = const_pool.tile([128, 128], _BF16, tag="ident", name="ident")
        masks.make_identity(nc, ident[:])
        ones_t = const_pool.tile([128, 1], _BF16, tag="ones", name="ones")
        nc.gpsimd.memset(ones_t[:], 1.0)

        # ---- DMA issue order = consumption order (HWDGE is in-order) -----
        wv_t = w_pool.tile([128, 2, 4, 2, 256], _FP8, tag="wv", name="wvt")
        nc.sync.dma_start(wv_t[:], wvr[:, :])
        xv_t = []
        for half in range(2):
            t = x_pool.tile([128, 2, 4, 2, 1024], _FP8, tag="xv",
                            name=f"xv{half}", bufs=2)
            nc.sync.dma_start(t[:], xvr[:, half * 16384:(half + 1) * 16384])
            xv_t.append(t)
        wkc_t = w_pool.tile([128, 2, 8, 2, 256], _FP8, tag="wkc", name="wkct")
        nc.sync.dma_start(wkc_t[:], wkc2[:, :])
        wkk_t = w_pool.tile([128, 2, 4, 2, 256], _FP8, tag="wkk", name="wkkt")
        nc.sync.dma_start(wkk_t[:], wkk2[:, :])
        wgc_t = const_pool.tile([128, 1], _BF16, tag="wgc", name="wgct")
        nc.sync.dma_start(wgc_t[:], wgc[:, :])
        pb_t = bias_pool.tile([128, 8], _FP32, tag="pb", name="pbt")
        nc.sync.dma_start(pb_t[:], pb[:, :])
        g0_t = bias_pool.tile([128, 1], _FP32, tag="g0", name="g0t")
        nc.sync.dma_start(g0_t[:], g0[:, :])

        SINV = float(1.0 / 256.0)  # undo the host fp8 pre-scales SX*SW

        def copy_bias(dst, src, bias_ap, on_act):
            """PSUM->SBUF copy: (src/256) + bias, on DVE or ACT."""
            if on_act:
                nc.scalar.activation(dst, src, _IDENT, bias=bias_ap,
                                     scale=SINV)
            else:
                nc.vector.tensor_scalar(dst, src, SINV, bias_ap,
                                        op0=mybir.AluOpType.mult,
                                        op1=mybir.AluOpType.add)

        qcat = [qk_pool.tile([128, L], _BF16, tag=f"qcat{h}", name=f"qcat{h}")
                for h in range(HPC)]
        kcat = [qk_pool.tile([128, L], _BF16, tag=f"kcat{h}", name=f"kcat{h}")
                for h in range(HPC)]
        vb4 = [None] * NJ
        sbq_t = [None] * NJ
        xq_t = [None] * NCH

        def load_sbq(rows):
            for r in rows:
                t = sb_pool.tile([128, L], _BF16, tag="sbq", name=f"sbq{r}")
                nc.sync.dma_start(t[:], sbq[r * 128:(r + 1) * 128, :])
                sbq_t[r] = t

        def load_xq(ch):
            """Two wide DMAs for p2q(ch): [q|pos_q] primary + residual."""
            tq = x_pool.tile([128, 2, 4, 2, 512], _FP8, tag="xq",
                             name=f"xq{ch}", bufs=4)
            nc.sync.dma_start(tq[:], xqr[:, ch * 16384:ch * 16384 + 8192])
            tp = x_pool.tile([128, 2, 4, 2, 512], _FP8, tag="xq",
                             name=f"xp{ch}", bufs=4)
            nc.sync.dma_start(tp[:],
                              xqr[:, ch * 16384 + 8192:(ch + 1) * 16384])
            xq_t[ch] = (tq, tp)

        # ---- PE warmup on scratch: ramp p-state while DMAs land ---------
        with tc.tile_pool(name="ps_warm", bufs=1, space="PSUM") as warm_pool:
            wmt = warm_pool.tile([128, 128], _FP32, tag="warm", name="warm")
            for i in range(NWARM):
                nc.tensor.matmul(wmt[:], scr[:, 0:128], scr[:, 0:128],
                                 start=True, stop=True)

        # compensated 3-term schedule: (x_half, w_half) in product order
        TERMS = ((0, 0), (0, 1), (1, 0))

        # ---- P1: v projection, compensated fp8 DoubleRow ----------------
        with tc.tile_pool(name="ps_v", bufs=8, space="PSUM") as psv_pool:
            for half in range(2):
                psv = [psv_pool.tile([128, CS], _FP32, tag="ps_v",
                                     name=f"psv{half}_{i}") for i in range(8)]
                xt = xv_t[half]
                for ti, (xh, wh) in enumerate(TERMS):
                    for kbp in range(KBP):
                        first = (ti == 0 and kbp == 0)
                        last = (ti == len(TERMS) - 1 and kbp == KBP - 1)
                        for i in range(8):
                            nc.tensor.matmul(
                                psv[i][:],
                                xt[:, xh, kbp, 0:2, i * 128:(i + 1) * 128],
                                wv_t[:, wh, kbp, 0:2, :],
                                start=first, stop=last, perf_mode=_DROW)
                for i in range(8):
                    tb = half * 8 + i
                    vb = v_pool.tile([128, CS], _BF16, tag=f"vb{tb}",
                                     name=f"vb{tb}")
                    nc.vector.tensor_scalar_mul(vb[:], psv[i][:], SINV)
                    vb4[tb] = vb

        # ---- P2: k-side projections, compensated DR, head-pair packed ---
        # kcat[even] = [kc+kp ; kc], kcat[odd] = [kc ; kc+kp]
        with tc.tile_pool(name="ps_k", bufs=8, space="PSUM") as psk_pool:
            for pc in range(2):  # key half (1024 keys each)
                ps1 = [[psk_pool.tile([128, 512], _FP32, tag="psk",
                                      name=f"k1_{pc}_{pr}_{i}")
                        for i in range(2)] for pr in range(2)]
                ps2 = [[psk_pool.tile([128, 512], _FP32, tag="psk",
                                      name=f"k2_{pc}_{pr}_{i}")
                        for i in range(2)] for pr in range(2)]
                for p in range(8):  # kb-pairs over the 2048-deep k||pos_k
                    t = x_pool.tile([128, 2, 2, 1024], _FP8, tag="xkk",
                                    name=f"xkk{pc}_{p}", bufs=5)
                    nc.sync.dma_start(
                        t[:], xkk[:, (pc * 8 + p) * 4096:
                                  (pc * 8 + p + 1) * 4096])
                    for pr in range(2):
                        psl = slice(pr * 128, (pr + 1) * 128)
                        for i in range(2):
                            isl = slice(i * 512, (i + 1) * 512)
                            for ti, (xh, wh) in enumerate(TERMS):
                                nc.tensor.matmul(
                                    ps1[pr][i][:],
                                    wkc_t[:, wh, p, 0:2, psl],
                                    t[:, xh, 0:2, isl],
                                    start=(p == 0 and ti == 0),
                                    stop=(p == 7 and ti == len(TERMS) - 1),
                                    perf_mode=_DROW)
                                if p < 4:
                                    nc.tensor.matmul(
                                        ps2[pr][i][:],
                                        wkk_t[:, wh, p, 0:2, psl],
                                        t[:, xh, 0:2, isl],
                                        start=(p == 0 and ti == 0),
                                        stop=(p == 3 and
                                              ti == len(TERMS) - 1),
                                        perf_mode=_DROW)
                if pc == 0:
                    wqp_t = w_pool.tile([128, 2, 4, 2, 2, 256], _FP8,
                                        tag="wqp", name="wqpt")
                    nc.sync.dma_start(wqp_t[:], wqp[:, :])
                for pr in range(2):
                    he, ho = 2 * pr, 2 * pr + 1
                    c = 4 + 2 * pr
                    for i in range(2):
                        csl = slice(pc * 1024 + i * 512,
                                    pc * 1024 + (i + 1) * 512)
                        on_act = i == 0
                        copy_bias(kcat[he][0:64, csl], ps1[pr][i][0:64, :],
                                  pb_t[0:64, c:c + 1], on_act)
                        copy_bias(kcat[ho][64:128, csl], ps1[pr][i][64:128, :],
                                  pb_t[64:128, c:c + 1], on_act)
                        copy_bias(kcat[ho][0:64, csl], ps2[pr][i][0:64, :],
                                  pb_t[0:64, c + 1:c + 2], not on_act)
                        copy_bias(kcat[he][64:128, csl], ps2[pr][i][64:128, :],
                                  pb_t[64:128, c + 1:c + 2], not on_act)

        load_xq(0)
        load_sbq(range(0, 4))
        wo_t = w_pool.tile([128, D], _BF16, tag="wo", name="wot")
        nc.sync.dma_start(wo_t[:], wo[0:128, :])
        wo_t2 = w_pool.tile([128, D], _BF16, tag="wo2", name="wot2")
        nc.sync.dma_start(wo_t2[:], wo[128:256, :])
        wo_ts = [wo_t, wo_t2]

        # ---- shared PSUM pools (8 banks exactly) -------------------------
        ps2_pool = pool(name="ps2", bufs=3, space="PSUM")
        pcv_pool = pool(name="ps_cv", bufs=1, space="PSUM")   # ctx accum
        pcs_pool = pool(name="ps_cs", bufs=1, space="PSUM")   # rowsums

        # ---- per-chunk pipeline ------------------------------------------
        def p2q_units(ch, pr, out):
            """p2q_pair split into filler units; `out` collects dg tiles."""

            def qproj(ti):
                xh, wh = TERMS[ti]

                def go():
                    csl = slice(ch * 512, (ch + 1) * 512)
                    prsl = slice(pr * 128, (pr + 1) * 128)
                    tq, tp = xq_t[ch]
                    if ti == 0:
                        out["d"] = ps2_pool.tile([128, 1024], _FP32,
                                                 tag="ps2", name=f"q{ch}{pr}")
                    d = out["d"]
                    for kbp in range(KBP):
                        st = (ti == 0 and kbp == 0)
                        sp = (ti == len(TERMS) - 1 and kbp == KBP - 1)
                        nc.tensor.matmul(d[:, 0:512],
                                         wqp_t[:, wh, kbp, 0, 0:2, prsl],
                                         tq[:, xh, kbp, 0:2, :],
                                         start=st, stop=sp, perf_mode=_DROW)
                        nc.tensor.matmul(d[:, 512:1024],
                                         wqp_t[:, wh, kbp, 1, 0:2, prsl],
                                         tp[:, xh, kbp, 0:2, :],
                                         start=st, stop=sp, perf_mode=_DROW)
                return go

            def tail():
                csl = slice(ch * 512, (ch + 1) * 512)
                d = out["d"]
                he, ho = 2 * pr, 2 * pr + 1
                c = 2 * pr
                copy_bias(qcat[he][0:64, csl], d[0:64, 0:512],
                          pb_t[0:64, c:c + 1], False)
                copy_bias(qcat[ho][64:128, csl], d[64:128, 0:512],
                          pb_t[64:128, c:c + 1], False)
                copy_bias(qcat[ho][0:64, csl], d[0:64, 512:1024],
                          pb_t[0:64, c + 1:c + 2], False)
                copy_bias(qcat[he][64:128, csl], d[64:128, 512:1024],
                          pb_t[64:128, c + 1:c + 2], False)
                for hh, h in enumerate((he, ho)):
                    rows = slice(0, 64) if h % 2 == 0 else slice(64, 128)
                    for qb in range(4):
                        nc.tensor.matmul(
                            d[:, hh * 4 + qb:hh * 4 + qb + 1],
                            qcat[h][rows, ch * 512 + qb * 128:
                                    ch * 512 + (qb + 1) * 128],
                            wgc_t[rows, 0:1], start=True, stop=True)
                g = gb_pool.tile([128, 8], _BF16, tag=f"gb{pr}",
                                 name=f"gb{pr}_{ch}", bufs=2)
                nc.scalar.activation(g[:], d[:, 0:8], _TANH,
                                     bias=g0_t[:, 0:1], scale=0.5)
                gp1 = gb_pool.tile([128, 8], _FP32, tag=f"gp{pr}",
                                   name=f"gp{pr}_{ch}", bufs=2)
                nc.vector.tensor_scalar_add(gp1[:], g[:], 1.0)
                dg = []
                for hh, h in enumerate((he, ho)):
                    ds = []
                    for sb in range(4):
                        d2 = dg_pool.tile([128, 128], _BF16,
                                          tag=f"dg{h}_{sb}",
                                          name=f"dg{h}_{sb}_{ch}", bufs=2)
                        nc.vector.tensor_scalar_mul(
                            d2[:], ident[:],
                            gp1[:, hh * 4 + sb:hh * 4 + sb + 1])
                        ds.append(d2)
                    dg.append(ds)
                out["dg"] = dg

            return [qproj(0), qproj(1), qproj(2), tail]

        def p2q_pair(ch, pr):
            """q-side projections for chunk ch, head pair pr, followed by the
            pair's column-gate/diag so A can start as soon as pair 0 lands."""
            csl = slice(ch * 512, (ch + 1) * 512)
            prsl = slice(pr * 128, (pr + 1) * 128)
            tq, tp = xq_t[ch]
            d = ps2_pool.tile([128, 1024], _FP32, tag="ps2", name=f"q{ch}{pr}")
            mq, mp = d[:, 0:512], d[:, 512:1024]
            for ti, (xh, wh) in enumerate(TERMS):
                for kbp in range(KBP):
                    st = (ti == 0 and kbp == 0)
                    sp = (ti == len(TERMS) - 1 and kbp == KBP - 1)
                    nc.tensor.matmul(mq, wqp_t[:, wh, kbp, 0, 0:2, prsl],
                                     tq[:, xh, kbp, 0:2, :],
                                     start=st, stop=sp, perf_mode=_DROW)
                    nc.tensor.matmul(mp, wqp_t[:, wh, kbp, 1, 0:2, prsl],
                                     tp[:, xh, kbp, 0:2, :],
                                     start=st, stop=sp, perf_mode=_DROW)
            he, ho = 2 * pr, 2 * pr + 1
            c = 2 * pr
            copy_bias(qcat[he][0:64, csl], d[0:64, 0:512],
                      pb_t[0:64, c:c + 1], False)
            copy_bias(qcat[ho][64:128, csl], d[64:128, 0:512],
                      pb_t[64:128, c:c + 1], False)
            copy_bias(qcat[ho][0:64, csl], d[0:64, 512:1024],
                      pb_t[0:64, c + 1:c + 2], False)
            copy_bias(qcat[he][64:128, csl], d[64:128, 512:1024],
                      pb_t[64:128, c + 1:c + 2], False)
            # column gate: 8 N=1 matmuls into a PSUM corner (free on PE),
            # one tiny tanh, diag blocks via per-partition tensor_scalar.
            for hh, h in enumerate((he, ho)):
                rows = slice(0, 64) if h % 2 == 0 else slice(64, 128)
                for qb in range(4):
                    nc.tensor.matmul(
                        d[:, hh * 4 + qb:hh * 4 + qb + 1],
                        qcat[h][rows, ch * 512 + qb * 128:
                                ch * 512 + (qb + 1) * 128],
                        wgc_t[rows, 0:1], start=True, stop=True)
            g = gb_pool.tile([128, 8], _BF16, tag=f"gb{pr}",
                             name=f"gb{pr}_{ch}", bufs=2)
            # gate' = 1 + tanh(z/2 + bg/2) = 2*sigmoid(z+bg); 0.5 folded
            # into spatial_bias host-side.
            nc.scalar.activation(g[:], d[:, 0:8], _TANH, bias=g0_t[:, 0:1],
                                 scale=0.5)
            gp1 = gb_pool.tile([128, 8], _FP32, tag=f"gp{pr}",
                               name=f"gp{pr}_{ch}", bufs=2)
            nc.vector.tensor_scalar_add(gp1[:], g[:], 1.0)
            dg = []
            for hh, h in enumerate((he, ho)):
                ds = []
                for sb in range(4):
                    d2 = dg_pool.tile([128, 128], _BF16, tag=f"dg{h}_{sb}",
                                      name=f"dg{h}_{sb}_{ch}", bufs=2)
                    nc.vector.tensor_scalar_mul(
                        d2[:], ident[:], gp1[:, hh * 4 + sb:hh * 4 + sb + 1])
                    ds.append(d2)
                dg.append(ds)
            return dg

        def attn_pass(ch, hp, dg, pcs, fillers=()):
            """Score + gated spatial bias + exp + ctx/rowsum accumulation
            for ONE head pair; `fillers` are small PE work units emitted one
            per j iteration to absorb the ACT exp-lag bubbles."""
            if hp == 0 and ch + 1 < NCH:
                load_xq(ch + 1)
                load_sbq(range(4 * (ch + 1), 4 * (ch + 2)))
            dsl = slice(ch * 512, (ch + 1) * 512)
            pcv = pcv_pool.tile([128, 512], _FP32, tag="ps_cv",
                                name=f"pcv{hp}_{ch}")
            fillers = list(fillers)
            for j in range(NJ):
                if j > 0 and fillers:
                    fillers.pop(0)()
                jsl = slice(j * 128, (j + 1) * 128)
                d = ps2_pool.tile([128, 1024], _FP32, tag="ps2", name="pss")
                for hh in range(2):
                    h = 2 * hp + hh
                    hof = hh * 512
                    nc.tensor.matmul(d[:, hof:hof + 512],
                                     kcat[h][:, jsl], qcat[h][:, dsl],
                                     start=True, stop=False)
                    for sb in range(4):
                        nc.tensor.matmul(
                            d[:, hof + sb * 128:hof + (sb + 1) * 128],
                            sbq_t[ch * 4 + sb][:, jsl],
                            dg[h - 2 * hp][sb][:],
                            start=False, stop=True)
                if j == 0:
                    # zero the accumulators AFTER j0's scores so the pass is
                    # not head-blocked on the previous pair's norm_div
                    nc.tensor.matmul(pcv[:], scr[:, 0:128], scr[:, 0:512],
                                     start=True, stop=False)
                    if hp == 0:
                        nc.tensor.matmul(pcs[:], scr[:, 0:128],
                                         scr[:, 0:16], start=True,
                                         stop=False)
                et = e_pool.tile([128, 1024], _BF16, tag="et", name="ett")
                nc.scalar.activation(et[:], d[:], _EXP)
                for hh in range(2):
                    h = 2 * hp + hh
                    for s in range(4):
                        esl = et[:, hh * 512 + s * 128:
                                 hh * 512 + (s + 1) * 128]
                        nc.tensor.matmul(
                            pcv[:, hh * 256 + s * 64:hh * 256 + (s + 1) * 64],
                            esl, vb4[j][:, h * 64:(h + 1) * 64],
                            start=False, stop=(j == NJ - 1))
                        nc.tensor.matmul(
                            pcs[:, h * 4 + s:h * 4 + s + 1],
                            esl, ones_t[:],
                            start=False, stop=(j == NJ - 1))
            for u in fillers:  # flush any unconsumed work units
                u()
            return pcv

        def norm_div(ch, pr, pcv, pcs):
            """Normalize one pair's ctx by its rowsums; frees pcv's bank."""
            inv = csb_pool.tile([128, 8], _FP32, tag=f"inv{pr}",
                                name=f"inv{pr}_{ch}")
            nc.vector.reciprocal(inv[:], pcs[:, pr * 8:(pr + 1) * 8])
            csb2 = {}
            for s in range(4):
                t = csb_pool.tile([128, 128], _BF16, tag=f"cs{pr}_{s}",
                                  name=f"cs{pr}_{s}_{ch}")
                for hh in range(2):
                    nc.vector.tensor_scalar_mul(
                        t[:, hh * 64:(hh + 1) * 64],
                        pcv[:, hh * 256 + s * 64:hh * 256 + (s + 1) * 64],
                        inv[:, hh * 4 + s:hh * 4 + s + 1])
                csb2[s] = t
            return csb2

        def cta_tile(ch, pr):
            return cta_pool.tile([128, 512], _BF16, tag=f"cta{pr}",
                                 name=f"cta{pr}_{ch}")

        def t_units(csb2, cta):
            """Transposes for one pair, 2 per ring double (one per bank)."""
            units = []

            def t_unit(s2):
                def go():
                    d16 = ps2_pool.tile([128, 2048], _BF16, tag="ps2",
                                        name="ptt")
                    for hh in range(2):
                        s = 2 * s2 + hh
                        nc.tensor.matmul(d16[:, hh * 1024:hh * 1024 + 128],
                                         csb2[s][:], ident[:],
                                         is_transpose=True)
                    for hh in range(2):
                        s = 2 * s2 + hh
                        nc.vector.tensor_copy(
                            cta[:, s * 128:(s + 1) * 128],
                            d16[:, hh * 1024:hh * 1024 + 128])
                return go

            for s2 in range(2):
                units.append(t_unit(s2))
            return units

        def o_units(ch, cta01):
            units = []

            def o_unit(op):
                def go():
                    d = ps2_pool.tile([128, 1024], _FP32, tag="ps2",
                                      name=f"o{ch}{op}")
                    for hh in range(2):
                        ob = 2 * op + hh
                        for kb in range(2):
                            nc.tensor.matmul(
                                d[:, hh * 512:(hh + 1) * 512],
                                wo_ts[kb][:, ob * 128:(ob + 1) * 128],
                                cta01[kb][:], start=(kb == 0), stop=(kb == 1))
                    ot = oute_pool.tile([128, 1024], _BF16, tag="ot",
                                        name="ott", bufs=2)
                    nc.vector.tensor_copy(ot[:], d[:])
                    for hh in range(2):
                        ob = 2 * op + hh
                        nc.sync.dma_start(
                            outT[ob * 128:(ob + 1) * 128,
                                 ch * 512:(ch + 1) * 512],
                            ot[:, hh * 512:(hh + 1) * 512])
                return go

            for op in range(KB_D // 2):
                units.append(o_unit(op))
            return units

        # pipeline: passA(ch) absorbs prev chunk's pair-1 transposes + O;
        # passB(ch) absorbs this chunk's pair-0 transposes.
        prev = None          # (ch, pcv_pair1, pcs, cta_pair0)
        nxt = None           # {"0": out-dict pair0, "1": out-dict pair1}
        for ch in range(NCH):
            if nxt is None:
                dg0 = p2q_pair(ch, 0)
            fillA = []
            if prev is not None:
                pch, ppcv1, ppcs, pcta0 = prev
                csb2p1 = norm_div(pch, 1, ppcv1, ppcs)
            if nxt is None:
                dg1 = p2q_pair(ch, 1)
            else:
                dg0, dg1 = nxt["0"]["dg"], nxt["1"]["dg"]
            if prev is not None:
                pcta1 = cta_tile(pch, 1)
                fillA = t_units(csb2p1, pcta1) + o_units(pch, [pcta0, pcta1])
            pcs = pcs_pool.tile([128, 16], _FP32, tag="pcs", name=f"pcs{ch}")
            pcv0 = attn_pass(ch, 0, dg0, pcs, fillA)
            csb2a = norm_div(ch, 0, pcv0, pcs)
            cta0 = cta_tile(ch, 0)
            fillB = t_units(csb2a, cta0)
            if ch + 1 < NCH:
                nxt = {"0": {}, "1": {}}
                fillB = (fillB + p2q_units(ch + 1, 0, nxt["0"])
                         + p2q_units(ch + 1, 1, nxt["1"]))
            pcv1 = attn_pass(ch, 1, dg1, pcs, fillB)
            prev = (ch, pcv1, pcs, cta0)
        pch, ppcv1, ppcs, pcta0 = prev
        csb2p1 = norm_div(pch, 1, ppcv1, ppcs)
        pcta1 = cta_tile(pch, 1)
        for u in t_units(csb2p1, pcta1) + o_units(pch, [pcta0, pcta1]):
            u()

    _split_multiwaits(nc)
    return nc


_NC_CACHE = {}


def _get_nc():
    if "nc" not in _NC_CACHE:
        _NC_CACHE["nc"] = build_nc()
    return _NC_CACHE["nc"]


def _np_reference(k, v, q, mask, spatial_bias, pos_k, pos_q,
                  Wk, bk, Wv, bv, Wq, bq, Wpk, bpk, Wpq, bpq, Wo, bo, Wg, bg):
    """Slow numpy fallback (only if mask is not all-True)."""
    def lin(x, W, b):
        return x @ W.T + b

    def split(x):
        return x.reshape(B, L, H, -1).transpose(0, 2, 1, 3)

    k_c, v_c, q_c = split(lin(k, Wk, bk)), split(lin(v, Wv, bv)), split(lin(q, Wq, bq))
    k_p, q_p = split(lin(pos_k, Wpk, bpk)), split(lin(pos_q, Wpq, bpq))
    scores = (np.einsum("bhqd,bhkd->bhqk", q_c, k_c)
              + np.einsum("bhqd,bhkd->bhqk", q_c, k_p)
              + np.einsum("bhqd,bhkd->bhqk", q_p, k_c)) * SCALE
    gate = 1.0 / (1.0 + np.exp(-(q_c @ Wg.T + bg)))
    scores = scores + gate * spatial_bias
    scores = np.where(mask[:, None, :, :], scores, -np.inf)
    scores = scores - scores.max(-1, keepdims=True)
    e = np.exp(scores)
    attn = e / e.sum(-1, keepdims=True)
    ctx = np.einsum("bhqk,bhkd->bhqd", attn, v_c)
    ctx = ctx.transpose(0, 2, 1, 3).reshape(B, L, D)
    return lin(ctx, Wo, bo).astype(np.float32)


def _pairstack8(w, scale):
    """[NP*256, M] fp32 -> ([128, NP*2*M] fp8 primary, same-shape residual):
    sub-major stacking of 128-row block PAIRS (DoubleRow K-tile layout).
    `scale` lifts the values out of fp8's subnormal floor so the residual
    (re-quantized to fp8) stays ~4% of a 4% correction; the joint x*w scale
    (SX*SW=256) is divided out in the PSUM->SBUF copies."""
    w = w * scale
    w8 = w.astype(FP8)
    dw8 = (w - w8.astype(np.float32)).astype(FP8)
    npair = w.shape[0] // 256
    m = w.shape[1]

    def stk(a):
        blocks = []
        for p in range(npair):
            blocks.append(np.stack(
                [a[(2 * p) * 128:(2 * p + 1) * 128],
                 a[(2 * p + 1) * 128:(2 * p + 2) * 128]],
                axis=1).reshape(128, 2 * m))
        return np.ascontiguousarray(np.concatenate(blocks, axis=1))

    return stk(w8), stk(dw8)


SX = 4.0    # x-stream fp8 pre-scale
SW = 64.0   # weight fp8 pre-scale
SINV = float(1.0 / (SX * SW))


def _comp(w, scale):
    """[NP*256, M] -> [128, 2*NP*2*M] fp8: primary block then residual."""
    a, b = _pairstack8(w, scale)
    return np.ascontiguousarray(np.concatenate([a, b], axis=1))


def kernel(k, v, q, mask, spatial_bias, pos_k, pos_q,
           Wk, bk, Wv, bv, Wq, bq, Wpk, bpk, Wpq, bpq, Wo, bo, Wg, bg,
           **_unused):
    f32 = lambda x: np.asarray(x, np.float32)
    k, v, q, pos_k, pos_q = f32(k), f32(v), f32(q), f32(pos_k), f32(pos_q)
    spatial_bias = f32(spatial_bias)
    mask = np.asarray(mask)
    Wk, Wv, Wq, Wpk, Wpq, Wo, Wg = map(f32, (Wk, Wv, Wq, Wpk, Wpq, Wo, Wg))
    bk, bv, bq, bpk, bpq, bo, bg = map(f32, (bk, bv, bq, bpk, bpq, bo, bg))

    if not mask.all():
        return _np_reference(k, v, q, mask, spatial_bias, pos_k, pos_q,
                             Wk, bk, Wv, bv, Wq, bq, Wpk, bpk, Wpq, bpq,
                             Wo, bo, Wg, bg)

    nc = _get_nc()

    xqr_b, xkk_b, xvr_b, sbq_b = [], [], [], []
    for b in range(B):
        qT, pT = q[b].T, pos_q[b].T                 # [D, L] fp32
        chunks = []
        for ch in range(NCH):
            csl = slice(ch * 512, (ch + 1) * 512)
            chunks.append(_comp(qT[:, csl], SX))
            chunks.append(_comp(pT[:, csl], SX))
        xqr_b.append(np.ascontiguousarray(np.concatenate(chunks, axis=1)))
        kstack = np.vstack([k[b].T, pos_k[b].T])    # [2D, L]
        xkk_b.append(np.ascontiguousarray(np.concatenate(
            [_xkk_interleave(kstack[:, pc * 1024:(pc + 1) * 1024])
             for pc in range(2)], axis=1)))
        vT = v[b].T
        xvr_b.append(np.ascontiguousarray(np.concatenate(
            [_comp(vT[:, half * 1024:(half + 1) * 1024], SX)
             for half in range(2)], axis=1)))
        # 0.5 folded from the tanh-gate: gate = 0.5*(1+tanh((z+bg)/2))
        sbq_b.append(np.ascontiguousarray(
            spatial_bias[b, 0] * 0.5).astype(BF16))

    WqT, WpqT = Wq.T * SCALE, Wpq.T * SCALE
    WkT, WpkT, WvT, WoT = Wk.T, Wpk.T, Wv.T, Wo.T
    in_maps = []
    for c in range(NCORES):
        b, g = c // 4, c % 4
        cs = slice(g * CS, (g + 1) * CS)

        def chs(hl):  # global channel slice for local head hl
            h = 4 * g + hl
            return slice(h * HK, (h + 1) * HK)

        wq2_a = np.empty((D, 256), np.float32)
        wp2_a = np.empty((D, 256), np.float32)
        wkc_a = np.empty((2 * D, 256), np.float32)
        wkk_a = np.empty((D, 256), np.float32)
        pb_a = np.empty((128, 8), np.float32)
        for pr in range(2):
            he, ho = 2 * pr, 2 * pr + 1
            pc = slice(pr * 128, pr * 128 + 64)
            pc2 = slice(pr * 128 + 64, pr * 128 + 128)
            wq2_a[:, pc] = WqT[:, chs(he)]
            wq2_a[:, pc2] = WqT[:, chs(ho)]
            wp2_a[:, pc] = WpqT[:, chs(ho)]      # swapped
            wp2_a[:, pc2] = WpqT[:, chs(he)]
            wkc_a[0:D, pc] = WkT[:, chs(he)]
            wkc_a[D:, pc] = WpkT[:, chs(he)]
            wkc_a[0:D, pc2] = WkT[:, chs(ho)]
            wkc_a[D:, pc2] = WpkT[:, chs(ho)]
            wkk_a[:, pc] = WkT[:, chs(ho)]       # swapped
            wkk_a[:, pc2] = WkT[:, chs(he)]
            # per-partition biases for the half-copies
            c0 = 2 * pr
            pb_a[0:64, c0] = bq[chs(he)] * SCALE
            pb_a[64:128, c0] = bq[chs(ho)] * SCALE
            pb_a[0:64, c0 + 1] = bpq[chs(ho)] * SCALE
            pb_a[64:128, c0 + 1] = bpq[chs(he)] * SCALE
            c4 = 4 + 2 * pr
            pb_a[0:64, c4] = (bk + bpk)[chs(he)]
            pb_a[64:128, c4] = (bk + bpk)[chs(ho)]
            pb_a[0:64, c4 + 1] = bk[chs(ho)]
            pb_a[64:128, c4 + 1] = bk[chs(he)]
        # wqp stripes: per (w_half, kb-pair): [qc pair block | qp pair block]
        wq8, dwq8 = _pairstack8(wq2_a, SW)    # [128, 4*512] each
        wp8, dwp8 = _pairstack8(wp2_a, SW)
        wqp_l = []
        for wh, (aq, ap_) in enumerate(((wq8, wp8), (dwq8, dwp8))):
            for p in range(KBP):
                wqp_l.append(aq[:, p * 512:(p + 1) * 512])
                wqp_l.append(ap_[:, p * 512:(p + 1) * 512])
        wqp_a = np.ascontiguousarray(np.concatenate(wqp_l, axis=1))
        in_maps.append({
            "xqr": xqr_b[b], "xkk": xkk_b[b], "xvr": xvr_b[b],
            "sbq": sbq_b[b],
            "wqp": wqp_a,
            "wkc2": _comp(wkc_a, SW),
            "wkk2": _comp(wkk_a, SW),
            "wvr": _comp(np.ascontiguousarray(WvT[:, cs]), SW),
            "wgc": np.tile((Wg[0] * (1.0 / SCALE)).astype(BF16), 2)
                     .reshape(128, 1),
            "wo": np.ascontiguousarray(WoT[cs, :]).astype(BF16),
            "pb": pb_a,
            "g0": np.full((128, 1), float(bg[0]) * 0.5, np.float32),
        })

    res = run_bass_kernel_spmd(nc, in_maps, core_ids=list(range(NCORES)))

    const_row = (bv @ WoT + bo).astype(np.float32)  # exact bv/bo fold
    out = np.empty((B, L, D), np.float32)
    for b in range(B):
        acc = res.results[b * 4]["outT"].astype(np.float32, copy=True)
        for g in range(1, 4):
            acc += res.results[b * 4 + g]["outT"]
        out[b] = acc.T + const_row
    return out


def _xkk_interleave(kst):
    """[2048, 1024] -> [128, 16384] fp8: per kb-pair p: [primary 2048 |
    residual 2048] so each stream tile holds both compensation halves."""
    a, r = _pairstack8(kst, SX)   # [128, 8*2048] each
    blocks = []
    for p in range(8):
        blocks.append(a[:, p * 2048:(p + 1) * 2048])
        blocks.append(r[:, p * 2048:(p + 1) * 2048])
    return np.ascontiguousarray(np.concatenate(blocks, axis=1))
